# revision 3
# baseline (speedup 1.0000x reference)
"""Trainium2 Bass kernel for the Neural ODE (dopri5, fixed substeps).

v2 -> v3: ALL inputs packed into ONE bf16 DRAM tensor ("blob").  Under
axon/PJRT each input-tensor handle costs ~2ms PER CALL (measured: 10
tiny inputs -> 21.5ms/call, 1 input -> 5.2ms/call), so the v1/v2 kernels
paid ~20ms/call just for buffer bindings.  fp32 quantities (x0, b2) ride
as bf16 hi/lo pairs and are reconstructed on device (exact to ~2^-16).

Also: dummy prologue tanh so the act-table load hoists out of the loop
(v2 reloaded the tanh table every iteration, 49 x 1.28us).

Blob layout ([NROWS, 3072] bf16), per core:
  rows 8k .. 8k+7, k in 0..NITER:  u interval k ([8, 24*BC]), k=NITER is
      a zero pad so the in-loop prefetch of interval i+1 stays in bounds
  R_W1  ..+72 : W1aug [73, 256] (bf16, cols 0:256)
  R_W2  ..+255: W2    [256, 64]
  R_M   ..+127: m_blk [128, 3072] (c-scaled blocks of W2 @ W1x)
  R_B2M       : b2m   [1, 1536]
  R_X0H ..+63 : x0T hi [64, BC]
  R_X0L ..+63 : x0T lo [64, BC]
  R_B2        : b2 hi row, b2 lo row  [1, 64] each
"""

import os
import numpy as np
import ml_dtypes

import concourse.bass as bass
import concourse.bacc as bacc
import concourse.mybir as mybir
import concourse.tile as tile
from concourse.bass_utils import run_bass_kernel_spmd
from concourse.bass_interp import get_hw_module

NCORES = int(os.environ.get('NODE_NCORES', 4))
B, D, F, H = 1024, 64, 8, 256
T, TU, N_SUB = 50, 128, 4
NITER = int(os.environ.get('NODE_NITER', T - 1))
BC = B // NCORES                   # 128 batch per core
KZ = D + F + 1                     # 73 = state + forcing + ones row
HH = H // 2                        # 128
NSLOT = N_SUB * 6                  # 24 stage slots per interval
ZC = NSLOT * BC                    # 3072 z_all columns

R_U = 0
R_W1 = 8 * (NITER + 1)
R_W2 = R_W1 + KZ
R_M = R_W2 + H
R_B2M = R_M + HH
R_X0H = R_B2M + 1
R_X0L = R_X0H + D
R_B2 = R_X0L + D
R_ONES = R_B2 + 2
NROWS = R_ONES + 1

f32 = mybir.dt.float32
bf16 = mybir.dt.bfloat16
FP = mybir.ActivationFunctionType
MULT = mybir.AluOpType.mult
ADD = mybir.AluOpType.add

A_TAB = [
    [],
    [1 / 5],
    [3 / 40, 9 / 40],
    [44 / 45, -56 / 15, 32 / 9],
    [19372 / 6561, -25360 / 2187, 64448 / 6561, -212 / 729],
    [9017 / 3168, -355 / 33, 46732 / 5247, 49 / 176, -5103 / 18656],
]
B_TAB = [35 / 384, 0.0, 500 / 1113, 125 / 192, -2187 / 6784, 11 / 84]

_CACHE = {}
LAST_RESULTS = None


def _host_times(t_eval):
    t_eval = np.asarray(t_eval, np.float32)
    dtc = np.diff(t_eval)
    frac = (np.arange(N_SUB, dtype=np.float32) / np.float32(N_SUB)).astype(
        np.float32)
    ts = (t_eval[:-1, None] + dtc[:, None] * frac).reshape(-1)
    dts = np.repeat(dtc / np.float32(N_SUB), N_SUB)
    return ts.astype(np.float32), dts.astype(np.float32)


def _stage_times(t, dt):
    t = np.float32(t)
    dt = np.float32(dt)
    return [
        t,
        t + dt / np.float32(5),
        t + np.float32(3) * dt / np.float32(10),
        t + np.float32(4) * dt / np.float32(5),
        t + np.float32(8) * dt / np.float32(9),
        t + dt,
    ]


def _split_hi_lo(w):
    w = np.asarray(w, np.float32)
    hi = w.astype(ml_dtypes.bfloat16)
    lo = (w - hi.astype(np.float32)).astype(ml_dtypes.bfloat16)
    return hi, lo


def _build_program(dt, b2_nonzero):
    nc = bacc.Bacc("TRN2", target_bir_lowering=False, debug=False,
                   enable_asserts=False)

    blob_d = nc.dram_tensor("blob", [NROWS, ZC], bf16, kind="ExternalInput")
    out_d = nc.dram_tensor("outT", [NITER, D, BC], f32, kind="ExternalOutput")

    with tile.TileContext(nc) as tc:
        with (
            tc.tile_pool(name="consts", bufs=1) as consts,
            tc.tile_pool(name="xs", bufs=3) as xs,
            tc.tile_pool(name="hs", bufs=2) as hs,
            tc.tile_pool(name="accs", bufs=12) as accs,
            tc.tile_pool(name="ph", bufs=2, space=bass.MemorySpace.PSUM) as ph,
            tc.tile_pool(name="php", bufs=1, space=bass.MemorySpace.PSUM) as php,
            tc.tile_pool(name="pf", bufs=2, space=bass.MemorySpace.PSUM) as pf,
        ):
            # --- persistent weights (all unpacked from the blob) ---
            w1_t = consts.tile([KZ, H], bf16, tag="w1")
            nc.sync.dma_start(out=w1_t[:], in_=blob_d[R_W1:R_W1 + KZ, 0:H])
            w2 = {}
            for half in range(2):
                t_ = consts.tile([HH, D], bf16, tag=f"w2{half}")
                r0 = R_W2 + half * HH
                nc.sync.dma_start(out=t_[:], in_=blob_d[r0:r0 + HH, 0:D])
                w2[half] = t_
            m_t = consts.tile([HH, 6, 2, 2, HH], bf16, tag="mblk")
            nc.sync.dma_start(out=m_t[:],
                              in_=blob_d[R_M:R_M + HH, 0:6 * 2 * 2 * HH])
            if b2_nonzero:
                ones_row = consts.tile([1, BC], bf16, tag="ones_row")
                nc.vector.memset(ones_row[:], 1.0)
                b2hi = consts.tile([1, D], bf16, tag="b2hi")
                b2lo = consts.tile([1, D], bf16, tag="b2lo")
                nc.sync.dma_start(out=b2hi[:], in_=blob_d[R_B2:R_B2 + 1, 0:D])
                nc.sync.dma_start(out=b2lo[:],
                                  in_=blob_d[R_B2 + 1:R_B2 + 2, 0:D])
                b2row_t = consts.tile([1, D], f32, tag="b2row")
                nc.vector.scalar_tensor_tensor(
                    out=b2row_t[:], in0=b2hi[:], scalar=1.0, in1=b2lo[:],
                    op0=MULT, op1=ADD)
                b2row_bf = consts.tile([1, D], bf16, tag="b2rowbf")
                nc.gpsimd.tensor_copy(out=b2row_bf[:], in_=b2row_t[:])
                b2m_t = consts.tile([1, 6, H], bf16, tag="b2m")
                nc.sync.dma_start(out=b2m_t[:],
                                  in_=blob_d[R_B2M:R_B2M + 1, 0:6 * H])

            # --- x0 = hi + lo ---
            x0h = consts.tile([D, BC], bf16, tag="x0h")
            x0l = consts.tile([D, BC], bf16, tag="x0l")
            nc.sync.dma_start(out=x0h[:], in_=blob_d[R_X0H:R_X0H + D, 0:BC])
            nc.sync.dma_start(out=x0l[:], in_=blob_d[R_X0L:R_X0L + D, 0:BC])
            xb = consts.tile([D, BC], f32, tag="xboundary")
            nc.vector.scalar_tensor_tensor(
                out=xb[:], in0=x0h[:], scalar=1.0, in1=x0l[:],
                op0=MULT, op1=ADD)

            # --- z_all: [73, 24*BC]; stage slot s occupies cols s*BC ---
            z_all = consts.tile([KZ, ZC], bf16, tag="z_all")
            # ones row arrives by DMA: engine writes must start at a
            # quarter-aligned partition, and row 72 is not
            nc.sync.dma_start(out=z_all[D + F:KZ, :],
                              in_=blob_d[R_ONES:R_ONES + 1, :])
            nc.sync.dma_start(out=z_all[D:D + F, :], in_=blob_d[0:F, :])
            nc.gpsimd.tensor_copy(out=z_all[0:D, 0:BC], in_=xb[0:D, :])

            # dummy tanh so the act table is loaded on the loop-entry path
            # (hoists the per-iteration LoadActFuncSet out of the loop);
            # read partition 0 (quarter-aligned) and give it a reader-free
            # sink that the verifier tolerates via the output DMA below
            dummy = consts.tile([1, 2], bf16, tag="dummy")
            nc.scalar.activation(dummy[:], z_all[0:1, 0:2], FP.Tanh)

            def zslot(s):
                return z_all[:, s * BC:(s + 1) * BC]

            def hp_accum(hp_next, slot, h_sb, ci):
                z_rhs = zslot(slot)
                for half in range(2):
                    sl = slice(half * HH, (half + 1) * HH)
                    nc.tensor.matmul(hp_next[:, half, 0:BC], w1_t[:, sl],
                                     z_rhs, start=True, stop=False)
                if b2_nonzero:
                    for half in range(2):
                        nc.tensor.matmul(
                            hp_next[:, half, 0:BC],
                            b2m_t[0:1, ci, half * HH:(half + 1) * HH],
                            ones_row[:], start=False, stop=False,
                            skip_group_check=True)
                for o in range(2):
                    for k in range(2):
                        nc.tensor.matmul(
                            hp_next[:, o, 0:BC], m_t[:, ci, k, o, :],
                            h_sb[:, k, :], start=False, stop=(k == 1))

            hp_b = php.tile([HH, 2, 512], f32, tag="hpb")
            for half in range(2):
                sl = slice(half * HH, (half + 1) * HH)
                nc.tensor.matmul(hp_b[:, half, 0:BC], w1_t[:, sl],
                                 zslot(0), start=True, stop=True)

            def axpy(eng, out, in0, scalar, in1):
                eng.scalar_tensor_tensor(out=out, in0=in0, scalar=scalar,
                                         in1=in1, op0=MULT, op1=ADD)

            def step_body(i, j, xT, hp_cur, boundary):
                base = j * 6
                nslot = 0 if boundary else base + 6

                nc.gpsimd.tensor_copy(out=z_all[0:D, (base + 1) * BC:
                                                (base + 2) * BC],
                                      in_=xT[0:D, :])

                acc = {tt: xT for tt in range(2, 6)}
                acc["xp"] = xT
                xT_new = None

                for st in range(6):
                    h_sb = hs.tile([HH, 2, BC], bf16, tag="h")
                    nc.scalar.activation(h_sb[:], hp_cur[:, :, 0:BC], FP.Tanh)

                    if st < 5:
                        hp_next = ph.tile([HH, 2, 512], f32, tag="hpre")
                        hp_accum(hp_next, base + st + 1, h_sb, st)
                    elif boundary:
                        hp_next = hp_b
                        hp_accum(hp_next, 0, h_sb, 5)
                    else:
                        hp_next = ph.tile([HH, 2, 512], f32, tag="hpre")
                        hp_accum(hp_next, nslot, h_sb, 5)

                    fp_t = pf.tile([D, BC], f32, tag="f")
                    for half in range(2):
                        nc.tensor.matmul(
                            fp_t[:], w2[half][:], h_sb[:, half, :],
                            start=(half == 0),
                            stop=(half == 1 and not b2_nonzero))
                    if b2_nonzero:
                        nc.tensor.matmul(fp_t[:], b2row_bf[:], ones_row[:],
                                         start=False, stop=True,
                                         skip_group_check=True)

                    for tt in range(st + 2, 6):
                        a = A_TAB[tt][st]
                        if a == 0.0:
                            continue
                        c = float(np.float64(a) * dt)
                        # all RK AXPYs read f from PSUM: GPSIMD cannot
                        # access PSUM, so they all live on DVE
                        eng = nc.vector
                        if st == tt - 2:
                            axpy(eng, z_all[0:D, (base + tt) * BC:
                                            (base + tt + 1) * BC],
                                 fp_t[:], c, acc[tt][0:D, :])
                        else:
                            nacc = accs.tile([D, BC], f32, tag="acc")
                            axpy(eng, nacc[:], fp_t[:], c, acc[tt][0:D, :])
                            acc[tt] = nacc
                    if B_TAB[st] != 0.0:
                        c = float(np.float64(B_TAB[st]) * dt)
                        if st == 4:
                            axpy(nc.vector, z_all[0:D, nslot * BC:
                                                  (nslot + 1) * BC],
                                 fp_t[:], c, acc["xp"][0:D, :])
                            nacc = accs.tile([D, BC], f32, tag="acc")
                            axpy(nc.vector, nacc[:], fp_t[:], c,
                                 acc["xp"][0:D, :])
                            acc["xp"] = nacc
                        elif st == 5:
                            xT_new = xb if boundary \
                                else xs.tile([D, BC], f32, tag="x")
                            axpy(nc.vector, xT_new[:], fp_t[:], c,
                                 acc["xp"][0:D, :])
                        else:
                            nacc = accs.tile([D, BC], f32, tag="acc")
                            axpy(nc.vector, nacc[:], fp_t[:], c,
                                 acc["xp"][0:D, :])
                            acc["xp"] = nacc

                    hp_cur = hp_next

                return xT_new, hp_cur

            with tc.For_i(0, NITER, 1) as i:
                xT, hp_cur = xb, hp_b
                for j in range(N_SUB):
                    xT, hp_cur = step_body(i, j, xT, hp_cur,
                                           boundary=(j == N_SUB - 1))
                    if j == 1:
                        # slots 0:12 fully read; prefetch interval i+1
                        nc.sync.dma_start(
                            out=z_all[D:D + F, 0:ZC // 2],
                            in_=blob_d[bass.ds(8 * i + 8, F), 0:ZC // 2])
                nc.sync.dma_start(
                    out=z_all[D:D + F, ZC // 2:ZC],
                    in_=blob_d[bass.ds(8 * i + 8, F), ZC // 2:ZC])
                nc.sync.dma_start(out=out_d[bass.ds(i, 1), :, :], in_=xb[:])

    nc.compile()
    return nc


def _prep_inputs(x0, t_eval, t_u, u_batch, W1, b1, W2, b2):
    ts, dts = _host_times(t_eval)
    nstep = NITER * N_SUB
    tq_all = np.empty((nstep, 6), np.float32)
    for s in range(nstep):
        tq_all[s] = _stage_times(ts[s], dts[s])
    tq_flat = tq_all.reshape(-1)
    idx = np.clip(np.searchsorted(t_u, tq_flat, side="right") - 1, 0, TU - 2)
    w = ((tq_flat - t_u[idx]) / (t_u[idx + 1] - t_u[idx])).astype(np.float32)
    u_tb = np.ascontiguousarray(u_batch.transpose(1, 2, 0))  # [TU, F, B]
    u0 = u_tb[idx]                                           # [S, F, B]
    ui = (u0 + w[:, None, None] * (u_tb[idx + 1] - u0)).astype(np.float32)
    u_all = ui.reshape(nstep, 6, F, B).transpose(0, 2, 1, 3)
    u_all = np.ascontiguousarray(
        u_all.reshape(NITER, N_SUB, F, 6, B)
        .transpose(0, 2, 1, 3, 4)
        .reshape(NITER, F, NSLOT, B)).astype(ml_dtypes.bfloat16)

    W1aug = np.concatenate([W1, b1[None, :]], axis=0).astype(
        ml_dtypes.bfloat16)                                  # [73, 256]
    w2b = W2.astype(ml_dtypes.bfloat16)

    dt64 = float(np.float64(dts).mean())
    MM = np.float64(W2) @ np.float64(W1[0:D, :])             # [256, 256]
    cs = [A_TAB[st + 1][st] * dt64 for st in range(5)] + [B_TAB[5] * dt64]
    m_blk = np.empty((HH, 6, 2, 2, HH), np.float32)
    b2m = np.empty((1, 6, H), np.float32)
    for ci, c in enumerate(cs):
        S = (c * MM).astype(np.float32)
        for k in range(2):
            for o in range(2):
                m_blk[:, ci, k, o, :] = S[k * HH:(k + 1) * HH,
                                          o * HH:(o + 1) * HH]
        b2m[0, ci, :] = c * (np.float64(b2) @ np.float64(W1[0:D, :]))
    m_blk = m_blk.astype(ml_dtypes.bfloat16).reshape(HH, 6 * 2 * 2 * HH)
    b2m = b2m.astype(ml_dtypes.bfloat16).reshape(1, 6 * H)
    x0h, x0l = _split_hi_lo(x0.T)                            # [64, B]
    b2h, b2l = _split_hi_lo(b2[None, :])                     # [1, 64]
    return dts, u_all, W1aug, w2b, m_blk, b2m, x0h, x0l, b2h, b2l


def _make_blob(core, prep):
    (dts, u_all, w1, w2b, m_blk, b2m, x0h, x0l, b2h, b2l) = prep
    bsl = slice(core * BC, (core + 1) * BC)
    blob = np.zeros((NROWS, ZC), ml_dtypes.bfloat16)
    blob[R_U:R_U + 8 * NITER].reshape(NITER, F, NSLOT * BC)[:] = \
        u_all[:, :, :, bsl].reshape(NITER, F, NSLOT * BC)
    blob[R_W1:R_W1 + KZ, 0:H] = w1
    blob[R_W2:R_W2 + H, 0:D] = w2b
    blob[R_M:R_M + HH, 0:6 * 2 * 2 * HH] = m_blk
    blob[R_B2M:R_B2M + 1, 0:6 * H] = b2m
    blob[R_X0H:R_X0H + D, 0:BC] = x0h[:, bsl]
    blob[R_X0L:R_X0L + D, 0:BC] = x0l[:, bsl]
    blob[R_B2:R_B2 + 1, 0:D] = b2h
    blob[R_B2 + 1:R_B2 + 2, 0:D] = b2l
    blob[R_ONES:R_ONES + 1, :] = 1.0
    return blob


def make_in_maps(x0, b2, prep):
    return [{"blob": _make_blob(c, prep)} for c in range(NCORES)]


def kernel(x0, t_eval, t_u, u_batch, W1, b1, W2, b2):
    x0 = np.asarray(x0, np.float32)
    t_eval = np.asarray(t_eval, np.float32)
    t_u = np.asarray(t_u, np.float32)
    u_batch = np.asarray(u_batch, np.float32)
    W1 = np.asarray(W1, np.float32)
    b1 = np.asarray(b1, np.float32)
    W2 = np.asarray(W2, np.float32)
    b2 = np.asarray(b2, np.float32)

    prep = _prep_inputs(x0, t_eval, t_u, u_batch, W1, b1, W2, b2)
    dts = prep[0]
    dt = float(np.float64(dts).mean())
    assert np.ptp(np.float64(dts)) <= 1e-4 * abs(dt) + 1e-12
    b2_nonzero = bool(np.any(b2 != 0.0))

    key = (dt, b2_nonzero)
    if key not in _CACHE:
        _CACHE[key] = _build_program(dt, b2_nonzero)
    nc = _CACHE[key]

    in_maps = make_in_maps(x0, b2, prep)

    trace = bool(int(os.environ.get("NODE_TRACE", "0")))
    old_m = nc.m
    nc.m = get_hw_module(nc.m)
    try:
        res = run_bass_kernel_spmd(nc, in_maps, list(range(NCORES)),
                                   trace=trace)
    finally:
        nc.m = old_m
    global LAST_RESULTS
    LAST_RESULTS = res

    out = np.empty((B, T, D), np.float32)
    out[:, 0, :] = x0
    for c in range(NCORES):
        bsl = slice(c * BC, (c + 1) * BC)
        out[bsl, 1:, :] = res.results[c]["outT"].transpose(2, 0, 1)
    return out


if __name__ == "__main__":
    import reference
    inputs = {k: np.asarray(v) for k, v in reference.setup_inputs().items()}
    got = kernel(**inputs)
    print("kernel output", got.shape, got.dtype)


# revision 4
# speedup vs baseline: 1.3830x; 1.3830x over previous
"""Trainium2 Bass kernel for the Neural ODE (dopri5, fixed substeps).

v2 -> v3: ALL inputs packed into ONE bf16 DRAM tensor ("blob").  Under
axon/PJRT each input-tensor handle costs ~2ms PER CALL (measured: 10
tiny inputs -> 21.5ms/call, 1 input -> 5.2ms/call), so the v1/v2 kernels
paid ~20ms/call just for buffer bindings.  fp32 quantities (x0, b2) ride
as bf16 hi/lo pairs and are reconstructed on device (exact to ~2^-16).

Also: dummy prologue tanh so the act-table load hoists out of the loop
(v2 reloaded the tanh table every iteration, 49 x 1.28us).

Blob layout ([NROWS, 3072] bf16), per core:
  rows 8k .. 8k+7, k in 0..NITER:  u interval k ([8, 24*BC]), k=NITER is
      a zero pad so the in-loop prefetch of interval i+1 stays in bounds
  R_W1  ..+72 : W1aug [73, 256] (bf16, cols 0:256)
  R_W2  ..+255: W2    [256, 64]
  R_M   ..+127: m_blk [128, 3072] (c-scaled blocks of W2 @ W1x)
  R_B2M       : b2m   [1, 1536]
  R_X0H ..+63 : x0T hi [64, BC]
  R_X0L ..+63 : x0T lo [64, BC]
  R_B2        : b2 hi row, b2 lo row  [1, 64] each
"""

import os
import numpy as np
import ml_dtypes

import concourse.bass as bass
import concourse.bacc as bacc
import concourse.mybir as mybir
import concourse.tile as tile
from concourse.bass_utils import run_bass_kernel_spmd
from concourse.bass_interp import get_hw_module

NCORES = int(os.environ.get('NODE_NCORES', 4))
B, D, F, H = 1024, 64, 8, 256
T, TU, N_SUB = 50, 128, 4
NITER = int(os.environ.get('NODE_NITER', T - 1))
BC = B // NCORES                   # 128 batch per core
KZ = D + F + 1                     # 73 = state + forcing + ones row
HH = H // 2                        # 128
NSLOT = N_SUB * 6                  # 24 stage slots per interval
ZC = NSLOT * BC                    # 3072 z_all columns

R_U = 0
R_M = 8 * (NITER + 1)
MCOLS = 6 * 2 * 2 * HH               # 3072


def _pack_layout():
    """Pack the small pieces into the m-block's unused columns (beside
    cols 0:MCOLS of rows R_M..R_M+HH) when ZC allows, else into their own
    rows.  Returns ({name: (row, col)}, nrows)."""
    pieces = [("w1", KZ, H), ("w2h0", HH, D), ("w2h1", HH, D),
              ("x0h", D, BC), ("x0l", D, BC), ("b2m", 1, 6 * H),
              ("b2h", 1, D), ("b2l", 1, D)]
    pos = {}
    cur = MCOLS
    next_row = R_M + HH
    for name, rows, cols in pieces:
        if rows <= HH and cur + cols <= ZC:
            pos[name] = (R_M, cur)
            cur += cols
        else:
            pos[name] = (next_row, 0)
            next_row += rows
    pos["ones"] = (next_row, 0)
    return pos, next_row + 1


POS, NROWS = _pack_layout()

f32 = mybir.dt.float32
bf16 = mybir.dt.bfloat16
FP = mybir.ActivationFunctionType
MULT = mybir.AluOpType.mult
ADD = mybir.AluOpType.add

A_TAB = [
    [],
    [1 / 5],
    [3 / 40, 9 / 40],
    [44 / 45, -56 / 15, 32 / 9],
    [19372 / 6561, -25360 / 2187, 64448 / 6561, -212 / 729],
    [9017 / 3168, -355 / 33, 46732 / 5247, 49 / 176, -5103 / 18656],
]
B_TAB = [35 / 384, 0.0, 500 / 1113, 125 / 192, -2187 / 6784, 11 / 84]

_CACHE = {}
LAST_RESULTS = None


def _host_times(t_eval):
    t_eval = np.asarray(t_eval, np.float32)
    dtc = np.diff(t_eval)
    frac = (np.arange(N_SUB, dtype=np.float32) / np.float32(N_SUB)).astype(
        np.float32)
    ts = (t_eval[:-1, None] + dtc[:, None] * frac).reshape(-1)
    dts = np.repeat(dtc / np.float32(N_SUB), N_SUB)
    return ts.astype(np.float32), dts.astype(np.float32)


def _stage_times(t, dt):
    t = np.float32(t)
    dt = np.float32(dt)
    return [
        t,
        t + dt / np.float32(5),
        t + np.float32(3) * dt / np.float32(10),
        t + np.float32(4) * dt / np.float32(5),
        t + np.float32(8) * dt / np.float32(9),
        t + dt,
    ]


def _split_hi_lo(w):
    w = np.asarray(w, np.float32)
    hi = w.astype(ml_dtypes.bfloat16)
    lo = (w - hi.astype(np.float32)).astype(ml_dtypes.bfloat16)
    return hi, lo


def _build_program(dt, b2_nonzero):
    nc = bacc.Bacc("TRN2", target_bir_lowering=False, debug=False,
                   enable_asserts=False)

    blob_d = nc.dram_tensor("blob", [NROWS, ZC], bf16, kind="ExternalInput")
    out_d = nc.dram_tensor("outT", [NITER, D, BC], f32, kind="ExternalOutput")

    with tile.TileContext(nc) as tc:
        with (
            tc.tile_pool(name="consts", bufs=1) as consts,
            tc.tile_pool(name="xs", bufs=3) as xs,
            tc.tile_pool(name="hs", bufs=2) as hs,
            tc.tile_pool(name="accs", bufs=12) as accs,
            tc.tile_pool(name="ph", bufs=2, space=bass.MemorySpace.PSUM) as ph,
            tc.tile_pool(name="php", bufs=1, space=bass.MemorySpace.PSUM) as php,
            tc.tile_pool(name="pf", bufs=2, space=bass.MemorySpace.PSUM) as pf,
        ):
            # --- persistent weights (all unpacked from the blob) ---
            def bslice(name, rows, cols):
                r, c = POS[name]
                return blob_d[r:r + rows, c:c + cols]

            w1_t = consts.tile([KZ, H], bf16, tag="w1")
            nc.sync.dma_start(out=w1_t[:], in_=bslice("w1", KZ, H))
            w2 = {}
            for half in range(2):
                t_ = consts.tile([HH, D], bf16, tag=f"w2{half}")
                nc.sync.dma_start(out=t_[:],
                                  in_=bslice(f"w2h{half}", HH, D))
                w2[half] = t_
            m_t = consts.tile([HH, 6, 2, 2, HH], bf16, tag="mblk")
            nc.sync.dma_start(out=m_t[:], in_=blob_d[R_M:R_M + HH, 0:MCOLS])
            if b2_nonzero:
                ones_row = consts.tile([1, BC], bf16, tag="ones_row")
                nc.vector.memset(ones_row[:], 1.0)
                b2hi = consts.tile([1, D], bf16, tag="b2hi")
                b2lo = consts.tile([1, D], bf16, tag="b2lo")
                nc.sync.dma_start(out=b2hi[:], in_=bslice("b2h", 1, D))
                nc.sync.dma_start(out=b2lo[:], in_=bslice("b2l", 1, D))
                b2row_t = consts.tile([1, D], f32, tag="b2row")
                nc.vector.scalar_tensor_tensor(
                    out=b2row_t[:], in0=b2hi[:], scalar=1.0, in1=b2lo[:],
                    op0=MULT, op1=ADD)
                b2row_bf = consts.tile([1, D], bf16, tag="b2rowbf")
                nc.gpsimd.tensor_copy(out=b2row_bf[:], in_=b2row_t[:])
                b2m_t = consts.tile([1, 6, H], bf16, tag="b2m")
                nc.sync.dma_start(out=b2m_t[:], in_=bslice("b2m", 1, 6 * H))

            # --- x0 = hi + lo ---
            x0h = consts.tile([D, BC], bf16, tag="x0h")
            x0l = consts.tile([D, BC], bf16, tag="x0l")
            nc.sync.dma_start(out=x0h[:], in_=bslice("x0h", D, BC))
            nc.sync.dma_start(out=x0l[:], in_=bslice("x0l", D, BC))
            xb = consts.tile([D, BC], f32, tag="xboundary")
            nc.vector.scalar_tensor_tensor(
                out=xb[:], in0=x0h[:], scalar=1.0, in1=x0l[:],
                op0=MULT, op1=ADD)

            # --- z_all: [73, 24*BC]; stage slot s occupies cols s*BC ---
            z_all = consts.tile([KZ, ZC], bf16, tag="z_all")
            # ones row arrives by DMA: engine writes must start at a
            # quarter-aligned partition, and row 72 is not
            r1, _ = POS["ones"]
            nc.sync.dma_start(out=z_all[D + F:KZ, :],
                              in_=blob_d[r1:r1 + 1, :])
            nc.sync.dma_start(out=z_all[D:D + F, :], in_=blob_d[0:F, :])
            nc.gpsimd.tensor_copy(out=z_all[0:D, 0:BC], in_=xb[0:D, :])

            # dummy tanh so the act table is loaded on the loop-entry path
            # (hoists the per-iteration LoadActFuncSet out of the loop);
            # read partition 0 (quarter-aligned) and give it a reader-free
            # sink that the verifier tolerates via the output DMA below
            dummy = consts.tile([1, 2], bf16, tag="dummy")
            nc.scalar.activation(dummy[:], z_all[0:1, 0:2], FP.Tanh)

            def zslot(s):
                return z_all[:, s * BC:(s + 1) * BC]

            def hp_accum(hp_next, slot, h_sb, ci):
                z_rhs = zslot(slot)
                for half in range(2):
                    sl = slice(half * HH, (half + 1) * HH)
                    nc.tensor.matmul(hp_next[:, half, 0:BC], w1_t[:, sl],
                                     z_rhs, start=True, stop=False)
                if b2_nonzero:
                    for half in range(2):
                        nc.tensor.matmul(
                            hp_next[:, half, 0:BC],
                            b2m_t[0:1, ci, half * HH:(half + 1) * HH],
                            ones_row[:], start=False, stop=False,
                            skip_group_check=True)
                for o in range(2):
                    for k in range(2):
                        nc.tensor.matmul(
                            hp_next[:, o, 0:BC], m_t[:, ci, k, o, :],
                            h_sb[:, k, :], start=False, stop=(k == 1))

            hp_b = php.tile([HH, 2, 512], f32, tag="hpb")
            for half in range(2):
                sl = slice(half * HH, (half + 1) * HH)
                nc.tensor.matmul(hp_b[:, half, 0:BC], w1_t[:, sl],
                                 zslot(0), start=True, stop=True)

            def axpy(eng, out, in0, scalar, in1):
                eng.scalar_tensor_tensor(out=out, in0=in0, scalar=scalar,
                                         in1=in1, op0=MULT, op1=ADD)

            def step_body(i, j, xT, hp_cur, boundary):
                base = j * 6
                nslot = 0 if boundary else base + 6

                nc.gpsimd.tensor_copy(out=z_all[0:D, (base + 1) * BC:
                                                (base + 2) * BC],
                                      in_=xT[0:D, :])

                acc = {tt: xT for tt in range(2, 6)}
                acc["xp"] = xT
                xT_new = None

                for st in range(6):
                    h_sb = hs.tile([HH, 2, BC], bf16, tag="h")
                    nc.scalar.activation(h_sb[:], hp_cur[:, :, 0:BC], FP.Tanh)

                    if st < 5:
                        hp_next = ph.tile([HH, 2, 512], f32, tag="hpre")
                        hp_accum(hp_next, base + st + 1, h_sb, st)
                    elif boundary:
                        hp_next = hp_b
                        hp_accum(hp_next, 0, h_sb, 5)
                    else:
                        hp_next = ph.tile([HH, 2, 512], f32, tag="hpre")
                        hp_accum(hp_next, nslot, h_sb, 5)

                    fp_t = pf.tile([D, BC], f32, tag="f")
                    for half in range(2):
                        nc.tensor.matmul(
                            fp_t[:], w2[half][:], h_sb[:, half, :],
                            start=(half == 0),
                            stop=(half == 1 and not b2_nonzero))
                    if b2_nonzero:
                        nc.tensor.matmul(fp_t[:], b2row_bf[:], ones_row[:],
                                         start=False, stop=True,
                                         skip_group_check=True)

                    for tt in range(st + 2, 6):
                        a = A_TAB[tt][st]
                        if a == 0.0:
                            continue
                        c = float(np.float64(a) * dt)
                        # all RK AXPYs read f from PSUM: GPSIMD cannot
                        # access PSUM, so they all live on DVE
                        eng = nc.vector
                        if st == tt - 2:
                            axpy(eng, z_all[0:D, (base + tt) * BC:
                                            (base + tt + 1) * BC],
                                 fp_t[:], c, acc[tt][0:D, :])
                        else:
                            nacc = accs.tile([D, BC], f32, tag="acc")
                            axpy(eng, nacc[:], fp_t[:], c, acc[tt][0:D, :])
                            acc[tt] = nacc
                    if B_TAB[st] != 0.0:
                        c = float(np.float64(B_TAB[st]) * dt)
                        if st == 4:
                            axpy(nc.vector, z_all[0:D, nslot * BC:
                                                  (nslot + 1) * BC],
                                 fp_t[:], c, acc["xp"][0:D, :])
                            nacc = accs.tile([D, BC], f32, tag="acc")
                            axpy(nc.vector, nacc[:], fp_t[:], c,
                                 acc["xp"][0:D, :])
                            acc["xp"] = nacc
                        elif st == 5:
                            xT_new = xb if boundary \
                                else xs.tile([D, BC], f32, tag="x")
                            axpy(nc.vector, xT_new[:], fp_t[:], c,
                                 acc["xp"][0:D, :])
                        else:
                            nacc = accs.tile([D, BC], f32, tag="acc")
                            axpy(nc.vector, nacc[:], fp_t[:], c,
                                 acc["xp"][0:D, :])
                            acc["xp"] = nacc

                    hp_cur = hp_next

                return xT_new, hp_cur

            with tc.For_i(0, NITER, 1) as i:
                xT, hp_cur = xb, hp_b
                for j in range(N_SUB):
                    xT, hp_cur = step_body(i, j, xT, hp_cur,
                                           boundary=(j == N_SUB - 1))
                    if j == 1:
                        # slots 0:12 fully read; prefetch interval i+1
                        nc.sync.dma_start(
                            out=z_all[D:D + F, 0:ZC // 2],
                            in_=blob_d[bass.ds(8 * i + 8, F), 0:ZC // 2])
                nc.sync.dma_start(
                    out=z_all[D:D + F, ZC // 2:ZC],
                    in_=blob_d[bass.ds(8 * i + 8, F), ZC // 2:ZC])
                nc.sync.dma_start(out=out_d[bass.ds(i, 1), :, :], in_=xb[:])

    nc.compile()
    return nc


def _prep_inputs(x0, t_eval, t_u, u_batch, W1, b1, W2, b2):
    ts, dts = _host_times(t_eval)
    nstep = NITER * N_SUB
    tq_all = np.empty((nstep, 6), np.float32)
    for s in range(nstep):
        tq_all[s] = _stage_times(ts[s], dts[s])
    tq_flat = tq_all.reshape(-1)
    idx = np.clip(np.searchsorted(t_u, tq_flat, side="right") - 1, 0, TU - 2)
    w = ((tq_flat - t_u[idx]) / (t_u[idx + 1] - t_u[idx])).astype(np.float32)
    u_tb = np.ascontiguousarray(u_batch.transpose(1, 2, 0))  # [TU, F, B]
    u0 = u_tb[idx]                                           # [S, F, B]
    ui = (u0 + w[:, None, None] * (u_tb[idx + 1] - u0)).astype(np.float32)
    u_all = ui.reshape(nstep, 6, F, B).transpose(0, 2, 1, 3)
    u_all = np.ascontiguousarray(
        u_all.reshape(NITER, N_SUB, F, 6, B)
        .transpose(0, 2, 1, 3, 4)
        .reshape(NITER, F, NSLOT, B)).astype(ml_dtypes.bfloat16)

    W1aug = np.concatenate([W1, b1[None, :]], axis=0).astype(
        ml_dtypes.bfloat16)                                  # [73, 256]
    w2b = W2.astype(ml_dtypes.bfloat16)

    dt64 = float(np.float64(dts).mean())
    MM = np.float64(W2) @ np.float64(W1[0:D, :])             # [256, 256]
    cs = [A_TAB[st + 1][st] * dt64 for st in range(5)] + [B_TAB[5] * dt64]
    m_blk = np.empty((HH, 6, 2, 2, HH), np.float32)
    b2m = np.empty((1, 6, H), np.float32)
    for ci, c in enumerate(cs):
        S = (c * MM).astype(np.float32)
        for k in range(2):
            for o in range(2):
                m_blk[:, ci, k, o, :] = S[k * HH:(k + 1) * HH,
                                          o * HH:(o + 1) * HH]
        b2m[0, ci, :] = c * (np.float64(b2) @ np.float64(W1[0:D, :]))
    m_blk = m_blk.astype(ml_dtypes.bfloat16).reshape(HH, 6 * 2 * 2 * HH)
    b2m = b2m.astype(ml_dtypes.bfloat16).reshape(1, 6 * H)
    x0h, x0l = _split_hi_lo(x0.T)                            # [64, B]
    b2h, b2l = _split_hi_lo(b2[None, :])                     # [1, 64]
    return dts, u_all, W1aug, w2b, m_blk, b2m, x0h, x0l, b2h, b2l


def _make_blob(core, prep):
    (dts, u_all, w1, w2b, m_blk, b2m, x0h, x0l, b2h, b2l) = prep
    bsl = slice(core * BC, (core + 1) * BC)
    blob = np.zeros((NROWS, ZC), ml_dtypes.bfloat16)
    blob[R_U:R_U + 8 * NITER].reshape(NITER, F, NSLOT * BC)[:] = \
        u_all[:, :, :, bsl].reshape(NITER, F, NSLOT * BC)
    blob[R_M:R_M + HH, 0:MCOLS] = m_blk

    def put(name, arr):
        r, c = POS[name]
        blob[r:r + arr.shape[0], c:c + arr.shape[1]] = arr

    put("w1", w1)
    put("w2h0", w2b[0:HH])
    put("w2h1", w2b[HH:H])
    put("b2m", b2m)
    put("x0h", x0h[:, bsl])
    put("x0l", x0l[:, bsl])
    put("b2h", b2h)
    put("b2l", b2l)
    r1, _ = POS["ones"]
    blob[r1:r1 + 1, :] = 1.0
    return blob


def make_in_maps(x0, b2, prep):
    return [{"blob": _make_blob(c, prep)} for c in range(NCORES)]


def kernel(x0, t_eval, t_u, u_batch, W1, b1, W2, b2):
    x0 = np.asarray(x0, np.float32)
    t_eval = np.asarray(t_eval, np.float32)
    t_u = np.asarray(t_u, np.float32)
    u_batch = np.asarray(u_batch, np.float32)
    W1 = np.asarray(W1, np.float32)
    b1 = np.asarray(b1, np.float32)
    W2 = np.asarray(W2, np.float32)
    b2 = np.asarray(b2, np.float32)

    prep = _prep_inputs(x0, t_eval, t_u, u_batch, W1, b1, W2, b2)
    dts = prep[0]
    dt = float(np.float64(dts).mean())
    assert np.ptp(np.float64(dts)) <= 1e-4 * abs(dt) + 1e-12
    b2_nonzero = bool(np.any(b2 != 0.0))

    key = (dt, b2_nonzero)
    if key not in _CACHE:
        _CACHE[key] = _build_program(dt, b2_nonzero)
    nc = _CACHE[key]

    in_maps = make_in_maps(x0, b2, prep)

    trace = bool(int(os.environ.get("NODE_TRACE", "0")))
    old_m = nc.m
    nc.m = get_hw_module(nc.m)
    try:
        res = run_bass_kernel_spmd(nc, in_maps, list(range(NCORES)),
                                   trace=trace)
    finally:
        nc.m = old_m
    global LAST_RESULTS
    LAST_RESULTS = res

    out = np.empty((B, T, D), np.float32)
    out[:, 0, :] = x0
    for c in range(NCORES):
        bsl = slice(c * BC, (c + 1) * BC)
        out[bsl, 1:, :] = res.results[c]["outT"].transpose(2, 0, 1)
    return out


if __name__ == "__main__":
    import reference
    inputs = {k: np.asarray(v) for k, v in reference.setup_inputs().items()}
    got = kernel(**inputs)
    print("kernel output", got.shape, got.dtype)
